# revision 1
# baseline (speedup 1.0000x reference)
"""KNN cluster kernel for Trainium2 (8 NeuronCores, one batch per core).

Computes, for each of N=8 batches independently: squared L2 distances between
queries coords2[:, n, :] (L2=4096) and references coords1[:, n, :] (L1=4096)
in C=64 dims, then the indices of the 16 nearest references per query
(ascending distance). Output matches torch_cluster.knn-style flattened
(clusters, batch_idx) of the jax reference.

The end-to-end call is dominated by the axon tunnel (~64MB/s, ~30ms/RPC
floor), not device compute (~0.4ms/core), so the design minimizes moved
bytes and per-call dispatch work:

  - Inputs are shipped as int24 fixed point (x * 1.5e6 truncated to a
    3-byte int): 12MB instead of 16MB, in ONE concatenated tensor.
    Device reassembles i32 = v*256 with three strided byte-lane copies +
    a memset, then one i32->f32 convert. Distance ordering is invariant
    to the uniform scale, so no rescale is needed. Measured effect of the
    quantization on the task: 7/524288 index flips (relerr 0.0014)
    standalone, vs the 2e-2 gate.
  - Per core: row norms on the scalar engine; one augmented matmul
    (KAUG=66 contraction: [Q|1|-q2] x [2X|-x2|1]) yields s = -dist^2;
    chunked top-8-of-512 + merge gives the 16 winners; winner positions
    are resolved to global indices on-device (one
    scalar_tensor_tensor(is_equal, mult, accum_out) per winner), so the
    only output is (4096,16) uint16 per core - 1MB total.
  - Host dispatch: the stock axon run_bass_kernel_spmd path rebuilds its
    jax.jit(shard_map(bass_exec)) closure per call (re-trace + XLA
    re-compile + donated zero buffers shipped per call; ~670ms/call). We
    AOT-compile the identical program ONCE (fast_dispatch_compile), keep
    device-resident dummy buffers for the output-name operands (the NEFF
    binds outputs to custom-call results; those operands are never read),
    and fetch without an intermediate block so the output shards stream
    back while the remote call drains. Falls back to
    bass_utils.run_bass_kernel_spmd on any failure.
"""

import sys

import numpy as np

sys.path.insert(0, "/opt/trn_rl_repo")

L = 4096  # L1 == L2
N = 8
C = 64
K = 16
P = 128  # partitions / queries per tile
NT = L // P  # 32 query tiles
XC = 8  # matmul moving chunks of 512
MM_N = L // XC  # 512
NCH = 8  # top-k chunking of the 4096-wide row
CHW = L // NCH  # 512
NCAND = NCH * 8  # 64 candidates per query
KAUG = C + 2  # 66: contraction with -q2 / -x2 rows folded in
NEG_INF = -1.0e30
PACK_SCALE = np.float32(1.5e6)  # |coord| < 5.59 keeps |v| < 2^23

_CACHE = {}


def _decode24(nc, pool, mybir, pk_ap, name):
    """DMA packed int24 input and reassemble f32 in SBUF.

    Byte lanes 1..3 of an i32 tile are filled from the packed stream
    (lane 0 zeroed), giving i32 = v*256; one scalar copy converts to f32.
    Returns the f32 tile AP [P, NT*C].
    """
    u8 = mybir.dt.uint8
    pk = pool.tile([P, NT * C * 3], u8, tag=f"{name}_pk")
    nc.sync.dma_start(
        pk[:].rearrange("p (t c) -> p t c", c=C * 3),
        pk_ap.rearrange("(t p) c -> p t c", p=P),
    )
    val = pool.tile([P, NT * C], mybir.dt.int32, tag=f"{name}_i32")
    vb = val[:].bitcast(u8).rearrange("p (n k) -> p n k", k=4)
    pb = pk[:].rearrange("p (n k) -> p n k", k=3)
    nc.vector.memset(vb[:, :, 0:1], 0)
    nc.scalar.copy(vb[:, :, 1:2], pb[:, :, 0:1])
    nc.scalar.copy(vb[:, :, 2:3], pb[:, :, 1:2])
    nc.scalar.copy(vb[:, :, 3:4], pb[:, :, 2:3])
    out = pool.tile([P, NT * C], mybir.dt.float32, tag=f"{name}_f32")
    nc.scalar.copy(out[:], val[:])
    return out[:]


def build_body(tc, qx_ap, idx_ap):
    from concourse import mybir, masks

    nc = tc.nc
    f32 = mybir.dt.float32
    i32 = mybir.dt.int32
    u16 = mybir.dt.uint16
    alu = mybir.AluOpType

    with (
        tc.tile_pool(name="const", bufs=1) as const_pool,
        tc.tile_pool(name="inp", bufs=1) as inp_pool,
        tc.tile_pool(name="aug", bufs=1) as aug_pool,
        tc.tile_pool(name="tpsum", bufs=2, space="PSUM") as tpsum_pool,
        tc.tile_pool(name="mpsum", bufs=4, space="PSUM") as mpsum_pool,
        tc.tile_pool(name="s", bufs=3) as s_pool,
        tc.tile_pool(name="small", bufs=2) as small_pool,
    ):
        ident = const_pool.tile([P, P], f32)
        masks.make_identity(nc, ident[:])

        # gather constants: candidate slot iota and per-slot chunk base
        posi_i = const_pool.tile([P, NCAND], i32)
        base_i = const_pool.tile([P, NCAND], i32)
        nc.gpsimd.iota(posi_i[:], [[1, NCAND]], channel_multiplier=0)
        nc.gpsimd.iota(base_i[:], [[CHW, NCH], [0, 8]], channel_multiplier=0)
        posi_f = const_pool.tile([P, NCAND], f32)
        base_f = const_pool.tile([P, NCAND], f32)
        nc.scalar.copy(posi_f[:], posi_i[:])
        nc.scalar.copy(base_f[:], base_i[:])

        q_sb = _decode24(nc, inp_pool, mybir, qx_ap[0:L, :], "q")
        x_sb = _decode24(nc, inp_pool, mybir, qx_ap[L : 2 * L, :], "x")

        sqd = inp_pool.tile([P, C], f32)
        q2 = inp_pool.tile([P, NT], f32)
        x2 = inp_pool.tile([P, NT], f32)

        q3 = q_sb.rearrange("p (t c) -> p t c", c=C)
        x3 = x_sb.rearrange("p (t c) -> p t c", c=C)

        # q2[p, t] = sum_c Q[t*128+p, c]^2 (scalar engine: square + accum)
        for t in range(NT):
            nc.scalar.activation(
                sqd[:],
                q_sb[:, t * C : (t + 1) * C],
                mybir.ActivationFunctionType.Square,
                accum_out=q2[:, t : t + 1],
            )
        for t in range(NT):
            nc.scalar.activation(
                sqd[:],
                x_sb[:, t * C : (t + 1) * C],
                mybir.ActivationFunctionType.Square,
                accum_out=x2[:, t : t + 1],
            )

        # Augmented pre-transpose layouts [P, NT*KAUG] (float32r):
        #   Q rows: [Q | 1 | -q2]      X rows: [2X | -x2 | 1]
        aug_q = aug_pool.tile([P, NT * KAUG], f32)
        aug_x = aug_pool.tile([P, NT * KAUG], f32)
        aq3 = aug_q[:].rearrange("p (t e) -> p t e", e=KAUG)
        ax3 = aug_x[:].rearrange("p (t e) -> p t e", e=KAUG)
        nc.scalar.copy(aq3[:, :, 0:C], q3)
        nc.any.memset(aq3[:, :, C : C + 1], 1.0)
        nc.scalar.mul(aq3[:, :, C + 1 : C + 2], q2[:].rearrange("p (t o) -> p t o", o=1), -1.0)
        nc.scalar.mul(ax3[:, :, 0:C], x3, 2.0)
        nc.scalar.mul(ax3[:, :, C : C + 1], x2[:].rearrange("p (t o) -> p t o", o=1), -1.0)
        nc.any.memset(ax3[:, :, C + 1 : C + 2], 1.0)

        # Transposed operands [KAUG, L] float32r via PE transpose
        qT = aug_pool.tile([KAUG, L], f32)
        xT = aug_pool.tile([KAUG, L], f32)
        for t in range(NT):
            pq = tpsum_pool.tile([KAUG, P], f32, tag="tps")
            nc.tensor.transpose(pq[:], aug_q[:, t * KAUG : (t + 1) * KAUG], ident[:])
            nc.scalar.copy(qT[:, t * P : (t + 1) * P], pq[:])
            px = tpsum_pool.tile([KAUG, P], f32, tag="tps")
            nc.tensor.transpose(px[:], aug_x[:, t * KAUG : (t + 1) * KAUG], ident[:])
            nc.scalar.copy(xT[:, t * P : (t + 1) * P], px[:])

        # Main loop: per 128-query tile, matmul + two-pass chunked top-16
        for t in range(NT):
            s_sb = s_pool.tile([P, L], f32, tag="s")
            for j in range(XC):
                ps = mpsum_pool.tile([P, MM_N], f32, tag="mm")
                nc.tensor.matmul(
                    ps[:],
                    lhsT=qT[:, t * P : (t + 1) * P],
                    rhs=xT[:, j * MM_N : (j + 1) * MM_N],
                    start=True,
                    stop=True,
                )
                nc.scalar.copy(s_sb[:, j * MM_N : (j + 1) * MM_N], ps[:])

            cand_v = small_pool.tile([P, NCAND], f32, tag="cand_v")
            cand2 = small_pool.tile([P, NCAND], f32, tag="cand2")
            ci_t = small_pool.tile([P, NCAND], u16, tag="ci")
            v16 = small_pool.tile([P, 16], f32, tag="v16")
            pos_t = small_pool.tile([P, 16], u16, tag="pos")
            for ch in range(NCH):
                nc.vector.max(
                    cand_v[:, ch * 8 : (ch + 1) * 8],
                    s_sb[:, ch * CHW : (ch + 1) * CHW],
                )
            for ch in range(NCH):
                nc.vector.max_index(
                    ci_t[:, ch * 8 : (ch + 1) * 8],
                    cand_v[:, ch * 8 : (ch + 1) * 8],
                    s_sb[:, ch * CHW : (ch + 1) * CHW],
                )
            nc.vector.max(v16[:, 0:8], cand_v[:])
            nc.vector.match_replace(cand2[:], v16[:, 0:8], cand_v[:], NEG_INF)
            nc.vector.max(v16[:, 8:16], cand2[:])
            nc.vector.max_index(pos_t[:, 0:8], v16[:, 0:8], cand_v[:])
            nc.vector.max_index(pos_t[:, 8:16], v16[:, 8:16], cand2[:])

            # resolve winner positions -> global indices, all on-device:
            # g = ci + chunk_base; idx_j = sum((posi == pos_j) * g)
            ci_f = small_pool.tile([P, NCAND], f32, tag="ci_f")
            g = small_pool.tile([P, NCAND], f32, tag="g")
            pos_f = small_pool.tile([P, 16], f32, tag="pos_f")
            scr = small_pool.tile([P, NCAND], f32, tag="scr")
            idx_f = small_pool.tile([P, 16], f32, tag="idx_f")
            idx_u = small_pool.tile([P, 16], u16, tag="idx_u")
            nc.scalar.copy(ci_f[:], ci_t[:])
            nc.scalar.copy(pos_f[:], pos_t[:])
            nc.vector.scalar_tensor_tensor(
                g[:], ci_f[:], 0.0, base_f[:], op0=alu.bypass, op1=alu.add
            )
            for j in range(K):
                nc.vector.scalar_tensor_tensor(
                    scr[:],
                    posi_f[:],
                    pos_f[:, j : j + 1],
                    g[:],
                    op0=alu.is_equal,
                    op1=alu.mult,
                    accum_out=idx_f[:, j : j + 1],
                )
            nc.scalar.copy(idx_u[:], idx_f[:])
            nc.sync.dma_start(idx_ap[t * P : (t + 1) * P, :], idx_u[:])


def _build_program():
    from concourse import bacc, mybir, tile

    nc = bacc.Bacc(
        "TRN2",
        target_bir_lowering=False,
        debug=False,
        enable_asserts=True,
        num_devices=N,
    )
    qx_dram = nc.dram_tensor("qx", [2 * L, C * 3], mybir.dt.uint8, kind="ExternalInput")
    idx_dram = nc.dram_tensor("idx", [L, K], mybir.dt.uint16, kind="ExternalOutput")

    with tile.TileContext(nc) as tc:
        build_body(tc, qx_dram.ap(), idx_dram.ap())

    nc.compile()
    return nc


def _get_nc():
    if "nc" not in _CACHE:
        _CACHE["nc"] = _build_program()
    return _CACHE["nc"]


_PACK_SRC = r"""
#include <stdint.h>
void pack24(const float* src, uint8_t* dst, long rows, long cols,
            long rowstride, float scale) {
    for (long r = 0; r < rows; ++r) {
        const float* s = src + r * rowstride;
        uint8_t* d = dst + r * cols * 3;
        for (long c = 0; c < cols; ++c) {
            int32_t v = (int32_t)(s[c] * scale);
            d[3 * c] = (uint8_t)(v & 0xFF);
            d[3 * c + 1] = (uint8_t)((v >> 8) & 0xFF);
            d[3 * c + 2] = (uint8_t)((v >> 16) & 0xFF);
        }
    }
}
"""


def _get_cpack():
    """Compile the one-pass packer (scale + trunc-cast + 24-bit strip) once;
    returns the ctypes function or None (numpy fallback)."""
    if "cpack" in _CACHE:
        return _CACHE["cpack"]
    fn = None
    try:
        import ctypes
        import hashlib
        import os
        import subprocess
        import tempfile

        h = hashlib.sha256(_PACK_SRC.encode()).hexdigest()[:16]
        so = os.path.join(tempfile.gettempdir(), f"knnpack24_{h}.so")
        if not os.path.exists(so):
            with tempfile.NamedTemporaryFile("w", suffix=".c", delete=False) as f:
                f.write(_PACK_SRC)
                csrc = f.name
            subprocess.run(
                ["gcc", "-O3", "-march=native", "-shared", "-fPIC", "-o", so + ".tmp", csrc],
                check=True, capture_output=True,
            )
            os.replace(so + ".tmp", so)
            os.unlink(csrc)
        lib = ctypes.CDLL(so)
        lib.pack24.argtypes = [
            ctypes.c_void_p, ctypes.c_void_p,
            ctypes.c_long, ctypes.c_long, ctypes.c_long, ctypes.c_float,
        ]
        lib.pack24.restype = None
        fn = lib.pack24
    except Exception:
        fn = None
    _CACHE["cpack"] = fn
    return fn


def _get_bufs():
    if "bufs" not in _CACHE:
        _CACHE["bufs"] = np.empty((N, 2 * L, C * 3), np.uint8)
    return _CACHE["bufs"]


def _pack_inputs(coords1, coords2):
    """-> (N*2L, C*3) u8 int24: per core n, rows [0:L)=queries, [L:2L)=refs."""
    out = _get_bufs()
    cpack = _get_cpack()
    if cpack is not None:
        for n in range(N):
            cpack(
                coords2.ctypes.data + n * C * 4, out[n, :L].ctypes.data,
                L, C, N * C, float(PACK_SCALE),
            )
            cpack(
                coords1.ctypes.data + n * C * 4, out[n, L:].ctypes.data,
                L, C, N * C, float(PACK_SCALE),
            )
    else:
        for d, src in ((0, coords2), (1, coords1)):
            v = (src * PACK_SCALE).astype(np.int32)
            vb = v.view(np.uint8).reshape(L, N, C, 4)
            for n in range(N):
                dst = out[n, :L] if d == 0 else out[n, L:]
                dst.reshape(L, C, 3)[...] = vb[:, n, :, :3]
    return out.reshape(N * 2 * L, C * 3)


def _get_runner():
    """AOT-compile the shard_map'd bass_exec dispatch once (same program
    run_bass_kernel_spmd builds per call under axon)."""
    if "runner" in _CACHE:
        return _CACHE["runner"]

    import warnings

    import jax
    from jax.sharding import Mesh, PartitionSpec, NamedSharding

    with warnings.catch_warnings():
        warnings.simplefilter("ignore")
        from jax.experimental.shard_map import shard_map

    from concourse import bass2jax, mybir

    nc = _get_nc()
    bass2jax.install_neuronx_cc_hook()

    partition_name = nc.partition_id_tensor.name if nc.partition_id_tensor else None
    in_names, out_names, out_avals = [], [], []
    for alloc in nc.m.functions[0].allocations:
        if not isinstance(alloc, mybir.MemoryLocationSet):
            continue
        name = alloc.memorylocations[0].name
        if alloc.kind == "ExternalInput":
            if name != partition_name:
                in_names.append(name)
        elif alloc.kind == "ExternalOutput":
            out_avals.append(
                jax.core.ShapedArray(tuple(alloc.tensor_shape), mybir.dt.np(alloc.dtype))
            )
            out_names.append(name)
    assert in_names == ["qx"] and out_names == ["idx"], (in_names, out_names)

    full_in_names = list(in_names) + list(out_names)
    if partition_name is not None:
        full_in_names.append(partition_name)

    devices = jax.devices()[:N]
    mesh = Mesh(np.asarray(devices), ("core",))

    def _body(*args):
        operands = list(args)
        if partition_name is not None:
            operands.append(bass2jax.partition_id_tensor())
        return tuple(
            bass2jax._bass_exec_p.bind(
                *operands,
                out_avals=tuple(out_avals),
                in_names=tuple(full_in_names),
                out_names=tuple(out_names),
                lowering_input_output_aliases=(),
                sim_require_finite=True,
                sim_require_nnan=True,
                nc=nc,
            )
        )

    n_all = len(in_names) + len(out_names)
    mapped = shard_map(
        _body,
        mesh=mesh,
        in_specs=(PartitionSpec("core"),) * n_all,
        out_specs=(PartitionSpec("core"),) * len(out_names),
        check_rep=False,
    )

    sh = NamedSharding(mesh, PartitionSpec("core"))
    dummy_outs = [
        jax.device_put(np.zeros((N * av.shape[0], *av.shape[1:]), av.dtype), sh)
        for av in out_avals
    ]
    jax.block_until_ready(dummy_outs)

    arg_shapes = [
        jax.ShapeDtypeStruct((N * 2 * L, C * 3), np.uint8, sharding=sh),
    ] + [jax.ShapeDtypeStruct(d.shape, d.dtype, sharding=sh) for d in dummy_outs]

    try:
        compiled = bass2jax.fast_dispatch_compile(
            lambda: jax.jit(mapped).lower(*arg_shapes).compile()
        )
    except Exception:
        compiled = jax.jit(mapped)  # plain cached jit still beats per-call rebuild

    def run(qx_cat):
        out = compiled(qx_cat, *dummy_outs)[0]
        for s in out.addressable_shards:
            s.data.copy_to_host_async()
        return np.asarray(out)

    _CACHE["runner"] = run
    return run


def _run_fallback(qx_cat):
    from concourse.bass_utils import run_bass_kernel_spmd

    nc = _get_nc()
    in_maps = [
        {"qx": qx_cat[n * 2 * L : (n + 1) * 2 * L]} for n in range(N)
    ]
    res = run_bass_kernel_spmd(nc, in_maps, core_ids=list(range(N)))
    return np.concatenate([r["idx"] for r in res.results], axis=0)


def kernel(coords1, coords2, k):
    coords1 = np.ascontiguousarray(np.asarray(coords1), dtype=np.float32)
    coords2 = np.ascontiguousarray(np.asarray(coords2), dtype=np.float32)
    assert int(k) == K, f"kernel hardcoded for k={K}, got {k}"
    assert coords1.shape == (L, N, C) and coords2.shape == (L, N, C)

    qx_cat = _pack_inputs(coords1, coords2)

    try:
        idx = _get_runner()(qx_cat)
    except Exception:
        _CACHE.pop("runner", None)
        idx = _run_fallback(qx_cat)

    idx = idx.reshape(N, L, K)
    # global_idx = local + n*L1 ; clusters = global_idx mod L2 == local (L1==L2)
    clusters = np.transpose(idx, (2, 1, 0)).astype(np.int32).reshape(-1)
    if "batch_idx" not in _CACHE:
        _CACHE["batch_idx"] = np.ascontiguousarray(
            np.broadcast_to(np.arange(N, dtype=np.int32), (K, L, N))
        ).reshape(-1)
    return clusters, _CACHE["batch_idx"]



# revision 2
# speedup vs baseline: 1.9428x; 1.9428x over previous
"""KNN cluster kernel for Trainium2 (8 NeuronCores, one batch per core).

Computes, for each of N=8 batches independently: the 16 nearest references
coords1[:, n, :] (L1=4096) for every query coords2[:, n, :] (L2=4096) in
C=64 dims, ascending distance, matching torch_cluster.knn-style flattened
(clusters, batch_idx) of the jax reference.

The end-to-end call is dominated by the axon tunnel (~50MB/s serialized,
~45ms pipeline-fill), so the design minimizes moved bytes:

  - Inputs ship as int8 fixed point (rint(x*23)+128 as uint8): 4MB total
    instead of 16MB f32. Device decodes with a single activation
    (Copy, scale, bias) per operand — no byte surgery.
  - Device computes coarse squared distances via one augmented matmul
    (KAUG=66: [Q*2^-4 | 2^-4 | -q2*2^-4] x [X*2^-3 | -x2*2^-4+idx*2^-8 |
    2^-4]) giving s = -d2*2^-8 + idx*2^-12. All values sit exactly on an
    f32 grid, and the idx*2^-12 term is a tie-break folded into the -x2
    column for free: every s in a row is distinct, so 3 rounds of
    max8/max_index8/match_replace yield the EXACT coarse top-24 candidate
    set per query. Only the candidate indices come back: (4096, 24) u16
    per core = 1.5MB total.
  - The host re-ranks the 24 candidates per query against the original
    f32 coords (C, ~64 dot products/query vs 4096 on device) and emits
    the exact top-16. Candidate-set membership is robust to int8
    quantization even though exact ordering is not: measured 10/524288
    mismatches (relerr 0.0022) vs the 2e-2 gate, all from f32
    rounding-order flips.
  - Fetch is per-shard async; refine for batch n runs while batch n+1
    streams back. Packing runs per-shard too, each put dispatched as
    soon as its shard is packed.
  - Host dispatch: AOT-compile the shard_map'd bass_exec once
    (fast_dispatch_compile), keep device-resident dummy buffers for the
    output-name operands. Falls back to bass_utils.run_bass_kernel_spmd
    on any failure.
"""

import sys

import numpy as np

sys.path.insert(0, "/opt/trn_rl_repo")

L = 4096  # L1 == L2
N = 8
C = 64
K = 16
M = 24  # candidates fetched per query
P = 128  # partitions / queries per tile
NT = L // P  # 32 query tiles
XC = 8  # matmul moving chunks of 512
MM_N = L // XC  # 512
KAUG = C + 2  # 66: contraction with const / -q2 / -x2 rows folded in
NEG_INF = -1.0e30
QSCALE = np.float32(23.0)  # |coord| <= 5.52 maps into [-127, 127]

_CACHE = {}


def build_body(tc, qx_ap, idx_ap):
    from concourse import mybir, masks

    nc = tc.nc
    f32 = mybir.dt.float32
    i32 = mybir.dt.int32
    u8 = mybir.dt.uint8
    u16 = mybir.dt.uint16
    alu = mybir.AluOpType
    ActF = mybir.ActivationFunctionType

    with (
        tc.tile_pool(name="const", bufs=1) as const_pool,
        tc.tile_pool(name="inp", bufs=1) as inp_pool,
        tc.tile_pool(name="aug", bufs=1) as aug_pool,
        tc.tile_pool(name="tpsum", bufs=2, space="PSUM") as tpsum_pool,
        tc.tile_pool(name="mpsum", bufs=4, space="PSUM") as mpsum_pool,
        tc.tile_pool(name="s", bufs=2) as s_pool,
        tc.tile_pool(name="small", bufs=2) as small_pool,
    ):
        ident = const_pool.tile([P, P], f32)
        masks.make_identity(nc, ident[:])

        # global row index l = t*128 + p as f32, scaled 2^-8 (tie-break term)
        it_i = const_pool.tile([P, NT], i32)
        nc.gpsimd.iota(it_i[:], [[P, NT]], channel_multiplier=1)
        it_s = const_pool.tile([P, NT], f32)
        nc.scalar.mul(it_s[:], it_i[:], 2.0**-8)

        # u8 inputs: rows [0:L) queries, [L:2L) references
        qu = inp_pool.tile([P, NT * C], u8)
        nc.sync.dma_start(
            qu[:].rearrange("p (t c) -> p t c", c=C),
            qx_ap[0:L, :].rearrange("(t p) c -> p t c", p=P),
        )
        xu = inp_pool.tile([P, NT * C], u8)
        nc.sync.dma_start(
            xu[:].rearrange("p (t c) -> p t c", c=C),
            qx_ap[L : 2 * L, :].rearrange("(t p) c -> p t c", p=P),
        )

        # Augmented pre-transpose layouts [P, NT*KAUG]:
        #   Q rows: [qv*2^-4 | 2^-4 | -q2*2^-4]
        #   X rows: [xv*2^-3 | -x2*2^-4 + l*2^-8 | 2^-4]
        # => s = lhs . rhs = -(d2)*2^-8 + l*2^-12, exact on the f32 grid,
        # distinct per reference l (tie-break), top band |s| < 2^12.
        aug_q = aug_pool.tile([P, NT * KAUG], f32)
        aug_x = aug_pool.tile([P, NT * KAUG], f32)
        aq3 = aug_q[:].rearrange("p (t e) -> p t e", e=KAUG)
        ax3 = aug_x[:].rearrange("p (t e) -> p t e", e=KAUG)
        qu3 = qu[:].rearrange("p (t c) -> p t c", c=C)
        xu3 = xu[:].rearrange("p (t c) -> p t c", c=C)
        nc.scalar.activation(aq3[:, :, 0:C], qu3, ActF.Copy, bias=-8.0, scale=2.0**-4)
        nc.scalar.activation(ax3[:, :, 0:C], xu3, ActF.Copy, bias=-16.0, scale=2.0**-3)

        # row sums of squares (scalar engine: square + accum), scaled grids
        sqd = inp_pool.tile([P, C], f32)
        q2 = inp_pool.tile([P, NT], f32)  # q2_v * 2^-8
        x2 = inp_pool.tile([P, NT], f32)  # x2_v * 2^-6
        for t in range(NT):
            nc.scalar.activation(
                sqd[:], aq3[:, t, 0:C], ActF.Square, accum_out=q2[:, t : t + 1]
            )
        for t in range(NT):
            nc.scalar.activation(
                sqd[:], ax3[:, t, 0:C], ActF.Square, accum_out=x2[:, t : t + 1]
            )

        nc.any.memset(aq3[:, :, C : C + 1], 2.0**-4)
        nc.scalar.mul(
            aq3[:, :, C + 1 : C + 2], q2[:].rearrange("p (t o) -> p t o", o=1), -16.0
        )
        nc.vector.scalar_tensor_tensor(
            ax3[:, :, C : C + 1],
            x2[:].rearrange("p (t o) -> p t o", o=1),
            -4.0,
            it_s[:].rearrange("p (t o) -> p t o", o=1),
            op0=alu.mult,
            op1=alu.add,
        )
        nc.any.memset(ax3[:, :, C + 1 : C + 2], 2.0**-4)

        # Transposed operands [KAUG, L] via PE transpose
        qT = aug_pool.tile([KAUG, L], f32)
        xT = aug_pool.tile([KAUG, L], f32)
        for t in range(NT):
            pq = tpsum_pool.tile([KAUG, P], f32, tag="tps")
            nc.tensor.transpose(pq[:], aug_q[:, t * KAUG : (t + 1) * KAUG], ident[:])
            nc.scalar.copy(qT[:, t * P : (t + 1) * P], pq[:])
            px = tpsum_pool.tile([KAUG, P], f32, tag="tps")
            nc.tensor.transpose(px[:], aug_x[:, t * KAUG : (t + 1) * KAUG], ident[:])
            nc.scalar.copy(xT[:, t * P : (t + 1) * P], px[:])

        # Main loop: per 128-query tile, matmul + exact top-24 extraction
        for t in range(NT):
            s0 = s_pool.tile([P, L], f32, tag="s0")
            for j in range(XC):
                ps = mpsum_pool.tile([P, MM_N], f32, tag="mm")
                nc.tensor.matmul(
                    ps[:],
                    lhsT=qT[:, t * P : (t + 1) * P],
                    rhs=xT[:, j * MM_N : (j + 1) * MM_N],
                    start=True,
                    stop=True,
                )
                nc.scalar.copy(s0[:, j * MM_N : (j + 1) * MM_N], ps[:])

            pos = small_pool.tile([P, M], u16, tag="pos")
            s1 = s_pool.tile([P, L], f32, tag="s1")
            va = small_pool.tile([P, 8], f32, tag="va")
            vb = small_pool.tile([P, 8], f32, tag="vb")
            vc = small_pool.tile([P, 8], f32, tag="vc")
            # round 0: top 1..8
            nc.vector.max(va[:], s0[:])
            nc.vector.max_index(pos[:, 0:8], va[:], s0[:])
            nc.vector.match_replace(s1[:], va[:], s0[:], NEG_INF)
            # round 1: top 9..16
            nc.vector.max(vb[:], s1[:])
            nc.vector.max_index(pos[:, 8:16], vb[:], s1[:])
            nc.vector.match_replace(s0[:], vb[:], s1[:], NEG_INF)
            # round 2: top 17..24
            nc.vector.max(vc[:], s0[:])
            nc.vector.max_index(pos[:, 16:24], vc[:], s0[:])

            nc.sync.dma_start(idx_ap[t * P : (t + 1) * P, :], pos[:])


def _build_program():
    from concourse import bacc, mybir, tile

    nc = bacc.Bacc(
        "TRN2",
        target_bir_lowering=False,
        debug=False,
        enable_asserts=True,
        num_devices=N,
    )
    qx_dram = nc.dram_tensor("qx", [2 * L, C], mybir.dt.uint8, kind="ExternalInput")
    idx_dram = nc.dram_tensor("idx", [L, M], mybir.dt.uint16, kind="ExternalOutput")

    with tile.TileContext(nc) as tc:
        build_body(tc, qx_dram.ap(), idx_dram.ap())

    nc.compile()
    return nc


def _get_nc():
    if "nc" not in _CACHE:
        _CACHE["nc"] = _build_program()
    return _CACHE["nc"]


_C_SRC = r"""
#include <stdint.h>
#include <math.h>
#define L 4096
#define NB 8
#define CD 64
#define KK 16
#define MM 24

void pack8(const float* src, uint8_t* dst, long rows, long cols,
           long rowstride, float scale) {
    for (long r = 0; r < rows; ++r) {
        const float* s = src + r * rowstride;
        uint8_t* d = dst + r * cols;
        for (long c = 0; c < cols; ++c) {
            int v = (int)lrintf(s[c] * scale);
            v = v < -127 ? -127 : (v > 127 ? 127 : v);
            d[c] = (uint8_t)(v + 128);
        }
    }
}

static float x2buf[L];

void refine(const float* c1, const float* c2, const uint16_t* cand,
            long n, int32_t* out) {
    for (long l = 0; l < L; ++l) {
        const float* xp = c1 + (l * NB + n) * CD;
        float a = 0.0f;
        for (int c = 0; c < CD; ++c) a += xp[c] * xp[c];
        x2buf[l] = a;
    }
    for (long q = 0; q < L; ++q) {
        const float* qp = c2 + (q * NB + n) * CD;
        float q2 = 0.0f;
        for (int c = 0; c < CD; ++c) q2 += qp[c] * qp[c];
        float dv[MM];
        int32_t iv[MM];
        const uint16_t* cp = cand + q * MM;
        for (int m = 0; m < MM; ++m) {
            int32_t ci = cp[m];
            const float* xp = c1 + ((long)ci * NB + n) * CD;
            float acc = 0.0f;
            for (int c = 0; c < CD; ++c) acc += qp[c] * xp[c];
            dv[m] = q2 + x2buf[ci] - 2.0f * acc;
            iv[m] = ci;
        }
        for (int m = 1; m < MM; ++m) {
            float d = dv[m];
            int32_t ix = iv[m];
            int j = m - 1;
            while (j >= 0 && (dv[j] > d || (dv[j] == d && iv[j] > ix))) {
                dv[j + 1] = dv[j];
                iv[j + 1] = iv[j];
                --j;
            }
            dv[j + 1] = d;
            iv[j + 1] = ix;
        }
        int32_t* op = out + q * NB + n;
        for (int k = 0; k < KK; ++k) op[(long)k * L * NB] = iv[k];
    }
}
"""


def _get_clib():
    """Compile the packer + refiner once; returns the ctypes lib or None."""
    if "clib" in _CACHE:
        return _CACHE["clib"]
    lib = None
    try:
        import ctypes
        import hashlib
        import os
        import subprocess
        import tempfile

        h = hashlib.sha256(_C_SRC.encode()).hexdigest()[:16]
        so = os.path.join(tempfile.gettempdir(), f"knnhost_{h}.so")
        if not os.path.exists(so):
            with tempfile.NamedTemporaryFile("w", suffix=".c", delete=False) as f:
                f.write(_C_SRC)
                csrc = f.name
            subprocess.run(
                [
                    "gcc", "-O3", "-march=native", "-ffast-math", "-funroll-loops",
                    "-shared", "-fPIC", "-o", so + ".tmp", csrc, "-lm",
                ],
                check=True, capture_output=True,
            )
            os.replace(so + ".tmp", so)
            os.unlink(csrc)
        lib = ctypes.CDLL(so)
        lib.pack8.argtypes = [
            ctypes.c_void_p, ctypes.c_void_p,
            ctypes.c_long, ctypes.c_long, ctypes.c_long, ctypes.c_float,
        ]
        lib.pack8.restype = None
        lib.refine.argtypes = [
            ctypes.c_void_p, ctypes.c_void_p, ctypes.c_void_p,
            ctypes.c_long, ctypes.c_void_p,
        ]
        lib.refine.restype = None
    except Exception:
        lib = None
    _CACHE["clib"] = lib
    return lib


def _get_bufs():
    if "bufs" not in _CACHE:
        _CACHE["bufs"] = np.empty((N, 2 * L, C), np.uint8)
    return _CACHE["bufs"]


def _pack_shard_np(coords1, coords2, out, n):
    for d, src in ((0, coords2), (1, coords1)):
        v = np.clip(np.rint(src[:, n, :] * QSCALE), -127, 127).astype(np.int32) + 128
        dst = out[:L] if d == 0 else out[L:]
        dst[...] = v.astype(np.uint8)


def _refine_np(coords1, coords2, cand, n, clusters):
    x = coords1[:, n, :].astype(np.float32)
    q = coords2[:, n, :].astype(np.float32)
    x2 = np.einsum("lc,lc->l", x, x)
    q2 = np.einsum("lc,lc->l", q, q)
    d = (q2[:, None] + x2[cand] - 2.0 * np.einsum("qc,qmc->qm", q, x[cand])).astype(
        np.float32
    )
    ordr = np.lexsort((cand, d), axis=1)[:, :K]
    got = np.take_along_axis(cand.astype(np.int64), ordr, axis=1)  # (L, K)
    clusters.reshape(K, L, N)[:, :, n] = got.T.astype(np.int32)


def _get_runner():
    """AOT-compile the shard_map'd bass_exec dispatch once (same program
    run_bass_kernel_spmd builds per call under axon)."""
    if "runner" in _CACHE:
        return _CACHE["runner"]

    import warnings

    import jax
    from jax.sharding import Mesh, PartitionSpec, NamedSharding

    with warnings.catch_warnings():
        warnings.simplefilter("ignore")
        from jax.experimental.shard_map import shard_map

    from concourse import bass2jax, mybir

    nc = _get_nc()
    bass2jax.install_neuronx_cc_hook()

    partition_name = nc.partition_id_tensor.name if nc.partition_id_tensor else None
    in_names, out_names, out_avals = [], [], []
    for alloc in nc.m.functions[0].allocations:
        if not isinstance(alloc, mybir.MemoryLocationSet):
            continue
        name = alloc.memorylocations[0].name
        if alloc.kind == "ExternalInput":
            if name != partition_name:
                in_names.append(name)
        elif alloc.kind == "ExternalOutput":
            out_avals.append(
                jax.core.ShapedArray(tuple(alloc.tensor_shape), mybir.dt.np(alloc.dtype))
            )
            out_names.append(name)
    assert in_names == ["qx"] and out_names == ["idx"], (in_names, out_names)

    full_in_names = list(in_names) + list(out_names)
    if partition_name is not None:
        full_in_names.append(partition_name)

    devices = jax.devices()[:N]
    mesh = Mesh(np.asarray(devices), ("core",))

    def _body(*args):
        operands = list(args)
        if partition_name is not None:
            operands.append(bass2jax.partition_id_tensor())
        return tuple(
            bass2jax._bass_exec_p.bind(
                *operands,
                out_avals=tuple(out_avals),
                in_names=tuple(full_in_names),
                out_names=tuple(out_names),
                lowering_input_output_aliases=(),
                sim_require_finite=True,
                sim_require_nnan=True,
                nc=nc,
            )
        )

    n_all = len(in_names) + len(out_names)
    mapped = shard_map(
        _body,
        mesh=mesh,
        in_specs=(PartitionSpec("core"),) * n_all,
        out_specs=(PartitionSpec("core"),) * len(out_names),
        check_rep=False,
    )

    sh = NamedSharding(mesh, PartitionSpec("core"))
    dummy_outs = [
        jax.device_put(np.zeros((N * av.shape[0], *av.shape[1:]), av.dtype), sh)
        for av in out_avals
    ]
    jax.block_until_ready(dummy_outs)

    arg_shapes = [
        jax.ShapeDtypeStruct((N * 2 * L, C), np.uint8, sharding=sh),
    ] + [jax.ShapeDtypeStruct(d.shape, d.dtype, sharding=sh) for d in dummy_outs]

    try:
        compiled = bass2jax.fast_dispatch_compile(
            lambda: jax.jit(mapped).lower(*arg_shapes).compile()
        )
    except Exception:
        compiled = jax.jit(mapped)  # plain cached jit still beats per-call rebuild

    def run(coords1, coords2):
        bufs = _get_bufs()
        clib = _get_clib()
        arrs = []
        # pack shard n, dispatch its put immediately (transfers pipeline)
        for n in range(N):
            if clib is not None:
                clib.pack8(
                    coords2.ctypes.data + n * C * 4, bufs[n, :L].ctypes.data,
                    L, C, N * C, float(QSCALE),
                )
                clib.pack8(
                    coords1.ctypes.data + n * C * 4, bufs[n, L:].ctypes.data,
                    L, C, N * C, float(QSCALE),
                )
            else:
                _pack_shard_np(coords1, coords2, bufs[n], n)
            arrs.append(jax.device_put(bufs[n], devices[n]))
        qx = jax.make_array_from_single_device_arrays((N * 2 * L, C), sh, arrs)
        out = compiled(qx, *dummy_outs)[0]
        shards = sorted(out.addressable_shards, key=lambda s: s.index[0].start)
        for s in shards:
            s.data.copy_to_host_async()
        clusters = np.empty(K * L * N, np.int32)
        for s in shards:
            n = s.index[0].start // L
            cand = np.ascontiguousarray(np.asarray(s.data))
            if clib is not None:
                clib.refine(
                    coords1.ctypes.data, coords2.ctypes.data, cand.ctypes.data,
                    n, clusters.ctypes.data,
                )
            else:
                _refine_np(coords1, coords2, cand.astype(np.int64), n, clusters)
        return clusters

    _CACHE["runner"] = run
    return run


def _run_fallback(coords1, coords2):
    from concourse.bass_utils import run_bass_kernel_spmd

    nc = _get_nc()
    bufs = _get_bufs()
    for n in range(N):
        _pack_shard_np(coords1, coords2, bufs[n], n)
    in_maps = [{"qx": bufs[n]} for n in range(N)]
    res = run_bass_kernel_spmd(nc, in_maps, core_ids=list(range(N)))
    clusters = np.empty(K * L * N, np.int32)
    for n in range(N):
        cand = np.asarray(res.results[n]["idx"]).astype(np.int64)
        _refine_np(coords1, coords2, cand, n, clusters)
    return clusters


def kernel(coords1, coords2, k):
    coords1 = np.ascontiguousarray(np.asarray(coords1), dtype=np.float32)
    coords2 = np.ascontiguousarray(np.asarray(coords2), dtype=np.float32)
    assert int(k) == K, f"kernel hardcoded for k={K}, got {k}"
    assert coords1.shape == (L, N, C) and coords2.shape == (L, N, C)

    try:
        clusters = _get_runner()(coords1, coords2)
    except Exception:
        _CACHE.pop("runner", None)
        clusters = _run_fallback(coords1, coords2)

    if "batch_idx" not in _CACHE:
        _CACHE["batch_idx"] = np.ascontiguousarray(
            np.broadcast_to(np.arange(N, dtype=np.int32), (K, L, N))
        ).reshape(-1)
    return clusters, _CACHE["batch_idx"]
